# revision 11
# baseline (speedup 1.0000x reference)
"""Trainium2 Bass kernel for nn_Encoder (R-GCN style message passing).

Math (faithful to the reference, including its s-major/f-major index mismatch):
    supports_ = concat_s(A[s] @ features)            # [N, S*F], cols k=s*F+f
    Vmat      = (W_comp @ W.transpose(1,0,2)).reshape(S*F, E)   # rows k=f*S+s
    out       = supports_ @ Vmat
Equivalent per-relation form with Q_eff[s][f, e] = Vmat[s*F + f, e]:
    H_s = features @ Q_eff[s]          # [N, E]  (tiny)
    out = sum_s A[s] @ H_s

Sharding: output rows n split across 8 cores (NS=1024 each); each core
contracts its A[:, n_shard, :] against replicated features/W/W_comp.

Device kernel (per core):
  - A-shard streamed from HBM as float8_e3m4 (1 B/elem -> ~33.5 MB), the
    sole large operand; this halves DMA vs fp16 and is the memory roofline.
  - Contraction rows k ordered m-major: chunk c = mc*S + s, so the H tile
    for m-chunk mc is produced just-in-time by ONE [32,128]x[32,128] PE
    matmul covering all 4 relations (stationary ft chunk, moving qcat).
  - Main loop: out.T[E, NS] += Hc.T @ A_chunk with Hc [128,32] fp16
    stationary and A [128,512] fp8e3 moving (mixed-dtype matmul; PE
    streams 1 col/cycle -> ~110 us, just above the ~95 us DMA floor).
Host does layout-only transforms + the e3m4/fp16 casts; final gather is
a concat of per-core outT.T blocks.
"""

import os
import numpy as np
import ml_dtypes

import concourse.bass as bass
import concourse.mybir as mybir
from concourse import bacc, bass_utils
from concourse.tile import TileContext
from concourse.tile_rust import add_dep_helper

S, N, F, E = 4, 8192, 32, 32
P = 128
N_CORES = 8
NS = N // N_CORES          # 1024 output rows per core
KTOT = S * N               # 32768 contraction rows
NCHUNK = KTOT // P         # 256 K-chunks of 128
JPB = int(os.environ.get("KJPB", "8"))   # K-chunks per DMA block (mult of S)
NBLK = NCHUNK // JPB       # DMA blocks
HPB = JPB // S             # H (m-chunk) tiles per block
NMC = N // P               # 64 m-chunks

# A dtype ('f8e3' | 'fp16') and H/hcat dtype ('fp16' | 'bf16' | 'f8e3').
ADT = os.environ.get("KDT", "f8e3")
HDT = os.environ.get("KHD", "fp16")
# Row-tiled H phase: pack 4 K=32 H-matmuls into the PE's 4 row groups via
# tile_position so they run concurrently. Off: H matmuls already pipeline
# at ~60 ns in bursts, so this buys little.
HTILE = os.environ.get("KHTILE", "0") == "1"
GQ = 4                     # m-chunks per row-tiled H group
NGRP = NMC // GQ           # 16 H groups

_DT_MAP = {
    "f8e3": (mybir.dt.float8e3, ml_dtypes.float8_e3m4),
    "fp16": (mybir.dt.float16, np.float16),
    "bf16": (mybir.dt.bfloat16, ml_dtypes.bfloat16),
}


def _build(adt_key, hdt_key):
    """Build + finalize the per-core Bass program (same program on all cores)."""
    dt_a, _ = _DT_MAP[adt_key]
    dt_h, _ = _DT_MAP[hdt_key]
    f32 = mybir.dt.float32
    abufs = int(os.environ.get("KABUFS", "6"))

    nc = bacc.Bacc("TRN2")
    atc = nc.dram_tensor("atc", [KTOT, NS], dt_a, kind="ExternalInput")
    # HTILE: featT[i*F + f, g*P + c] = features[(g*GQ+i)*P + c, f] -- the 4
    # row-group copies stacked on partitions. Else plain features.T [F, N].
    FTP = GQ * F if HTILE else F           # featT partitions
    featT = nc.dram_tensor(
        "featT", [FTP, N // (GQ if HTILE else 1)], mybir.dt.float16,
        kind="ExternalInput",
    )
    # wpk cols: [wb0 | wb1 | wc0 | wc1], each [F, 128] with
    # wb_b[f, s*E+e] = W[b, (s*F+f)//S, e], wc_b[f, s*E+e] = W_comp[(s*F+f)%S, b]
    # (HTILE: rows tiled 4x on partitions to feed the 4 row groups.)
    wpk = nc.dram_tensor("wpk", [FTP, 4 * S * E], f32, kind="ExternalInput")
    outT = nc.dram_tensor("outT", [E, NS], f32, kind="ExternalOutput")

    # Contraction rows permuted so partition p's block data is one contiguous
    # run of JPB*NS bytes: row k = b*(P*JPB) + p*JPB + j.
    atc_r = atc.rearrange("(b p j) n -> b p (j n)", p=P, j=JPB)

    with TileContext(nc) as tc:
        with (
            tc.tile_pool(name="consts", bufs=1) as consts,
            tc.tile_pool(name="hcatp", bufs=12) as hcatp,
            tc.tile_pool(name="abuf", bufs=abufs) as apool,
            tc.tile_pool(name="hps", bufs=6, space="PSUM") as hps,
            tc.tile_pool(name="ops", bufs=1, space="PSUM") as opsum,
            tc.tile_pool(name="osb", bufs=1) as osb,
        ):
            # ---- constants first (tiny, on sync ring) ----
            ft = consts.tile([F, N], mybir.dt.float16)
            nc.sync.dma_start(ft, featT[:, :])
            wp = consts.tile([F, 4 * S * E], f32)
            nc.sync.dma_start(wp, wpk[:, :])

            # Each A block is split in half across the two HWDGE rings
            # (SP/sync and ACT/scalar) so both rings stream every block
            # concurrently. The sync/scalar queues carry ONLY dma_starts:
            # a dma_start parked on its abuf WAR semaphore would stall any
            # copy queued behind it (FIFO), gating the PE.
            HCOL = JPB * NS // 2

            def a_dma(b, ab):
                nc.scalar.dma_start(ab[:, :HCOL], atc_r[b][:, :HCOL])
                nc.sync.dma_start(ab[:, HCOL:], atc_r[b][:, HCOL:])

            pre = {}
            for b in range(min(4, NBLK)):
                ab = apool.tile([P, JPB * NS], dt_a)
                a_dma(b, ab)
                pre[b] = ab

            # ---- qcat [32, 128]: qcat[f, s*E+e] = V[s*F+f, e] ----
            qtmp = consts.tile([F, S * E], f32)
            qf = consts.tile([F, S * E], f32)
            nc.vector.tensor_mul(qtmp, wp[:, 0 : S * E], wp[:, 2 * S * E : 3 * S * E])
            nc.vector.tensor_mul(qf, wp[:, S * E : 2 * S * E], wp[:, 3 * S * E :])
            nc.vector.tensor_add(qf, qf, qtmp)
            qcat = consts.tile([F, S * E], mybir.dt.float16)
            nc.vector.tensor_copy(qcat, qf)

            # ---- main loop: H tiles just-in-time + streaming A matmul ----
            ps0 = opsum.tile([E, 512], f32)
            ps1 = opsum.tile([E, 512], f32)

            hct = {}

            def emit_h(mc):
                # H4 tile [128 m, (s,e)]: all 4 relations for m-chunk mc in one
                # matmul: ft[:, mc*P:(mc+1)*P].T @ qcat. Copies on vector only
                # (GPSIMD can't read PSUM; sync/scalar queues are DMA-only).
                hp = hps.tile([P, S * E], f32)
                mm = nc.tensor.matmul(
                    hp, ft[:, mc * P : (mc + 1) * P], qcat, start=True, stop=True
                )
                t = hcatp.tile([P, S * E], dt_h)
                nc.vector.tensor_copy(t, hp)
                hct[mc] = t
                return mm

            # H tiles are produced one block ahead of their consumers, and the
            # consumer block's first main matmul is order-pinned after the
            # H matmuls of block b+1: the hcat cast then lands while block b's
            # mains stream, instead of adding ~400 ns at every block boundary.
            for h in range(HPB):
                emit_h(h)
            for b in range(NBLK):
                if b in pre:
                    ab = pre.pop(b)
                else:
                    ab = apool.tile([P, JPB * NS], dt_a)
                    a_dma(b, ab)
                last_hmm = None
                if b + 1 < NBLK:
                    for h in range(HPB):
                        last_hmm = emit_h((b + 1) * HPB + h)
                for j in range(JPB):
                    c = b * JPB + j
                    s, h = j % S, j // S
                    hc = hct[b * HPB + h][:, s * E : (s + 1) * E]
                    first = c == 0
                    last = c == NCHUNK - 1
                    mm = nc.tensor.matmul(
                        ps0, hc, ab[:, j * NS : j * NS + 512],
                        start=first, stop=last, skip_group_check=True,
                    )
                    if j == 0 and last_hmm is not None:
                        add_dep_helper(mm.ins, last_hmm.ins, sync=False,
                                       reason="H prefetch before mains")
                    nc.tensor.matmul(
                        ps1, hc, ab[:, j * NS + 512 : (j + 1) * NS],
                        start=first, stop=last, skip_group_check=True,
                    )
                for h in range(HPB):
                    del hct[b * HPB + h]

            # split output halves across engines + both HWDGE rings
            ot0 = osb.tile([E, 512], f32, tag="ot0")
            ot1 = osb.tile([E, 512], f32, tag="ot1")
            nc.vector.tensor_copy(ot0, ps0)
            nc.vector.tensor_copy(ot1, ps1)
            nc.sync.dma_start(outT[:, 0:512], ot0)
            nc.scalar.dma_start(outT[:, 512:NS], ot1)

    nc.finalize()
    return nc


_built_cache = {}


def _get_nc(adt_key, hdt_key):
    key = (adt_key, hdt_key)
    if key not in _built_cache:
        _built_cache[key] = _build(adt_key, hdt_key)
    return _built_cache[key]


def _shard_inputs(features, A, W, W_comp, adt_key):
    _, np_a = _DT_MAP[adt_key]
    features = np.asarray(features, dtype=np.float32)
    A = np.asarray(A, dtype=np.float32)
    W = np.asarray(W, dtype=np.float32)
    W_comp = np.asarray(W_comp, dtype=np.float32)

    featT = np.ascontiguousarray(features.T).astype(np.float16)

    # wpk blocks, replicating the reference's (s*F+f) <-> (f*S+s) pairing
    kmat = np.arange(S * E).reshape(S, E)          # kmat[s, f] = s*F + f
    wb = W[:, kmat // S, :]                        # [2, s, f, e]
    wc = W_comp[kmat % S, :]                       # [s, f, b]
    blocks = [
        wb[0].transpose(1, 0, 2).reshape(F, S * E),
        wb[1].transpose(1, 0, 2).reshape(F, S * E),
        np.repeat(wc[:, :, 0].T[:, :, None], E, axis=2).reshape(F, S * E),
        np.repeat(wc[:, :, 1].T[:, :, None], E, axis=2).reshape(F, S * E),
    ]
    wpk = np.ascontiguousarray(np.concatenate(blocks, axis=1)).astype(np.float32)

    # One cast of the full A, then per-core byte gathers.
    A8 = A.astype(np_a)                            # [S, n, m]
    AT = np.ascontiguousarray(A8.transpose(0, 2, 1))   # [S, m, n]

    in_maps = []
    for c in range(N_CORES):
        blk = AT[:, :, c * NS : (c + 1) * NS]      # [S, N(m), NS]
        # rows k = b*(P*JPB) + p*JPB + (h*S + s), m = (b*HPB + h)*P + p
        at2 = blk.reshape(S, NBLK, HPB, P, NS).transpose(1, 3, 2, 0, 4)
        atc = np.ascontiguousarray(at2).reshape(KTOT, NS)
        in_maps.append({"atc": atc, "featT": featT, "wpk": wpk})
    return in_maps


def _run(features, A, W, W_comp, dt_key=None, trace=False):
    adt_key = dt_key or ADT
    nc = _get_nc(adt_key, HDT)
    in_maps = _shard_inputs(features, A, W, W_comp, adt_key)
    res = bass_utils.run_bass_kernel_spmd(
        nc, in_maps, core_ids=list(range(N_CORES)), trace=trace
    )
    out = np.concatenate(
        [res.results[c]["outT"].T for c in range(N_CORES)], axis=0
    ).astype(np.float32)
    return out, res


def kernel(features, A, W, W_comp):
    try:
        out, _ = _run(features, A, W, W_comp)
    except Exception:
        # Rare transient device-unrecoverable flakes: reset jax backends and
        # retry once with a freshly built program.
        import jax
        try:
            jax.clear_caches()
            jax.extend.backend.clear_backends()
        except Exception:
            pass
        _built_cache.clear()
        out, _ = _run(features, A, W, W_comp)
    return out


# revision 13
# speedup vs baseline: 1.0410x; 1.0410x over previous
"""Trainium2 Bass kernel for nn_Encoder (R-GCN style message passing).

Math (faithful to the reference, including its s-major/f-major index mismatch):
    supports_ = concat_s(A[s] @ features)            # [N, S*F], cols k=s*F+f
    Vmat      = (W_comp @ W.transpose(1,0,2)).reshape(S*F, E)   # rows k=f*S+s
    out       = supports_ @ Vmat
Equivalent per-relation form with Q_eff[s][f, e] = Vmat[s*F + f, e]:
    H_s = features @ Q_eff[s]          # [N, E]  (tiny)
    out = sum_s A[s] @ H_s

Sharding: output rows n split across 8 cores (NS=1024 each); each core
contracts its A[:, n_shard, :] against replicated features/W/W_comp.

Device kernel (per core):
  - A-shard streamed from HBM as float8_e3m4 (1 B/elem -> ~33.5 MB), the
    sole large operand; this halves DMA vs fp16 and sets the memory floor.
  - Contraction rows k ordered m-major (chunk c = mc*S + s) so each block's
    4 relations share one H tile.
  - H tiles [128, S*E] are produced by [32,128]x[32,128] PE matmuls (all 4
    relations at once) in 4 large groups of 16, batched to amortize the
    expensive fp16<->fp8 PE pipeline transitions (~460 ns per burst).
    Group 0 runs during the initial A-block DMA wait; later groups are
    emitted 16 blocks ahead, order-pinned before the consuming mains.
  - Main loop: out.T[E, NS] += Hc.T @ A_chunk with Hc [128,32] fp16
    stationary and A [128,512] fp8e3 moving (mixed-dtype matmul, 1 col/cyc
    -> ~110 us PE, just above the ~95 us DMA floor).
  - sync/scalar engine queues carry ONLY dma_starts (a dma_start parked on
    its buffer-WAR semaphore stalls everything behind it); all PSUM->SBUF
    copies run on vector (GPSIMD cannot read PSUM).
Host does layout-only transforms + the e3m4/fp16 casts; final gather is
a concat of per-core outT.T blocks.
"""

import os
import numpy as np
import ml_dtypes

import concourse.bass as bass
import concourse.mybir as mybir
from concourse import bacc, bass_utils
from concourse.tile import TileContext
from concourse.tile_rust import add_dep_helper

S, N, F, E = 4, 8192, 32, 32
P = 128
N_CORES = 8
NS = N // N_CORES          # 1024 output rows per core
KTOT = S * N               # 32768 contraction rows
NCHUNK = KTOT // P         # 256 K-chunks of 128
JPB = int(os.environ.get("KJPB", "4"))   # K-chunks per DMA block (= S)
NBLK = NCHUNK // JPB       # DMA blocks (m-chunks per block = JPB/S)
MPB = JPB // S             # m-chunks per block
NMC = N // P               # 64 m-chunks
GH = int(os.environ.get("KGH", "16"))    # m-chunks per H group
NG = NMC // GH             # H groups

# A dtype ('f8e3' | 'fp16') and H/hcat dtype ('fp16' | 'bf16').
ADT = os.environ.get("KDT", "f8e3")
HDT = os.environ.get("KHD", "fp16")

_DT_MAP = {
    "f8e3": (mybir.dt.float8e3, ml_dtypes.float8_e3m4),
    "fp16": (mybir.dt.float16, np.float16),
    "bf16": (mybir.dt.bfloat16, ml_dtypes.bfloat16),
}


def _build(adt_key, hdt_key):
    """Build + finalize the per-core Bass program (same program on all cores)."""
    dt_a, _ = _DT_MAP[adt_key]
    dt_h, _ = _DT_MAP[hdt_key]
    f32 = mybir.dt.float32
    abufs = int(os.environ.get("KABUFS", "8"))

    nc = bacc.Bacc("TRN2")
    atc = nc.dram_tensor("atc", [KTOT, NS], dt_a, kind="ExternalInput")
    featT = nc.dram_tensor("featT", [F, N], mybir.dt.float16, kind="ExternalInput")
    # wpk cols: [wb0 | wb1 | wc0 | wc1], each [F, 128] with
    # wb_b[f, s*E+e] = W[b, (s*F+f)//S, e], wc_b[f, s*E+e] = W_comp[(s*F+f)%S, b]
    wpk = nc.dram_tensor("wpk", [F, 4 * S * E], f32, kind="ExternalInput")
    outT = nc.dram_tensor("outT", [E, NS], f32, kind="ExternalOutput")

    # Contraction rows permuted so partition p's block data is one contiguous
    # run of JPB*NS bytes: row k = b*(P*JPB) + p*JPB + j  (j == relation s).
    atc_r = atc.rearrange("(b p j) n -> b p (j n)", p=P, j=JPB)

    with TileContext(nc) as tc:
        with (
            tc.tile_pool(name="consts", bufs=1) as consts,
            tc.tile_pool(name="hcatp", bufs=3) as hcatp,
            tc.tile_pool(name="abuf", bufs=abufs) as apool,
            tc.tile_pool(name="hps", bufs=4, space="PSUM") as hps,
            tc.tile_pool(name="ops", bufs=1, space="PSUM") as opsum,
            tc.tile_pool(name="osb", bufs=1) as osb,
        ):
            # ---- constants first on the sync ring (starts ~2 us before the
            # scalar ring); ft split so H group 0 starts on its first half.
            wp = consts.tile([F, 4 * S * E], f32)
            nc.sync.dma_start(wp, wpk[:, :])
            ft = consts.tile([F, N], mybir.dt.float16)
            nc.sync.dma_start(ft[:, : N // 2], featT[:, : N // 2])
            nc.sync.dma_start(ft[:, N // 2 :], featT[:, N // 2 :])

            def a_dma(b, ab):
                eng = nc.sync if b % 2 == 0 else nc.scalar
                eng.dma_start(ab, atc_r[b])

            pre = {}
            for b in range(min(4, NBLK)):
                ab = apool.tile([P, JPB * NS], dt_a)
                a_dma(b, ab)
                pre[b] = ab

            # ---- qcat [32, 128]: qcat[f, s*E+e] = V[s*F+f, e] ----
            qtmp = consts.tile([F, S * E], f32)
            qf = consts.tile([F, S * E], f32)
            nc.vector.tensor_mul(qtmp, wp[:, 0 : S * E], wp[:, 2 * S * E : 3 * S * E])
            nc.vector.tensor_mul(qf, wp[:, S * E : 2 * S * E], wp[:, 3 * S * E :])
            nc.vector.tensor_add(qf, qf, qtmp)
            qcat = consts.tile([F, S * E], dt_h)
            nc.vector.tensor_copy(qcat, qf)

            # ---- H group: GH m-chunks per SBUF tile, 4 H-matmuls packed per
            # PSUM bank, one cast per bank. hct[g][:, lmc*128 + s*32 :+32] is
            # the main-matmul stationary for m-chunk 16g+lmc, relation s.
            QPG = GH // 4                  # psum banks per group
            hct = {}

            def emit_group(g):
                gt = hcatp.tile([P, GH * S * E], dt_h)
                last = None
                for q in range(QPG):
                    hp = hps.tile([P, 4 * S * E], f32)
                    for i in range(4):
                        mc = g * GH + q * 4 + i
                        last = nc.tensor.matmul(
                            hp[:, i * S * E : (i + 1) * S * E],
                            ft[:, mc * P : (mc + 1) * P],
                            qcat, start=True, stop=True,
                        )
                    nc.vector.tensor_copy(
                        gt[:, q * 4 * S * E : (q + 1) * 4 * S * E], hp
                    )
                hct[g] = gt
                return last

            ps0 = opsum.tile([E, 512], f32)
            ps1 = opsum.tile([E, 512], f32)

            emit_group(0)
            bpg = NBLK // NG               # blocks per group
            for b in range(NBLK):
                if b in pre:
                    ab = pre.pop(b)
                else:
                    ab = apool.tile([P, JPB * NS], dt_a)
                    a_dma(b, ab)
                last_hmm = None
                if b % bpg == bpg // 2 and b // bpg + 1 < NG:
                    last_hmm = emit_group(b // bpg + 1)
                g, lmc = divmod(b * MPB, GH)
                gt = hct[g]
                for j in range(JPB):
                    c = b * JPB + j
                    s = j % S
                    hc = gt[:, ((lmc + j // S) * S + s) * E : ((lmc + j // S) * S + s + 1) * E]
                    first = c == 0
                    last = c == NCHUNK - 1
                    mm = nc.tensor.matmul(
                        ps0, hc, ab[:, j * NS : j * NS + 512],
                        start=first, stop=last, skip_group_check=True,
                    )
                    if j == 0 and last_hmm is not None:
                        add_dep_helper(mm.ins, last_hmm.ins, sync=False,
                                       reason="H group before mains")
                    nc.tensor.matmul(
                        ps1, hc, ab[:, j * NS + 512 : (j + 1) * NS],
                        start=first, stop=last, skip_group_check=True,
                    )

            # split output halves across both HWDGE rings; copies on vector
            ot0 = osb.tile([E, 512], f32, tag="ot0")
            ot1 = osb.tile([E, 512], f32, tag="ot1")
            nc.vector.tensor_copy(ot0, ps0)
            nc.vector.tensor_copy(ot1, ps1)
            nc.sync.dma_start(outT[:, 0:512], ot0)
            nc.scalar.dma_start(outT[:, 512:NS], ot1)

    nc.finalize()
    return nc


_built_cache = {}


def _get_nc(adt_key, hdt_key):
    key = (adt_key, hdt_key)
    if key not in _built_cache:
        _built_cache[key] = _build(adt_key, hdt_key)
    return _built_cache[key]


def _shard_inputs(features, A, W, W_comp, adt_key):
    _, np_a = _DT_MAP[adt_key]
    features = np.asarray(features, dtype=np.float32)
    A = np.asarray(A, dtype=np.float32)
    W = np.asarray(W, dtype=np.float32)
    W_comp = np.asarray(W_comp, dtype=np.float32)

    featT = np.ascontiguousarray(features.T).astype(np.float16)

    # wpk blocks, replicating the reference's (s*F+f) <-> (f*S+s) pairing
    kmat = np.arange(S * F).reshape(S, F)          # kmat[s, f] = s*F + f
    wb = W[:, kmat // S, :]                        # [2, s, f, e]
    wc = W_comp[kmat % S, :]                       # [s, f, b]
    blocks = [
        wb[0].transpose(1, 0, 2).reshape(F, S * E),
        wb[1].transpose(1, 0, 2).reshape(F, S * E),
        np.repeat(wc[:, :, 0].T[:, :, None], E, axis=2).reshape(F, S * E),
        np.repeat(wc[:, :, 1].T[:, :, None], E, axis=2).reshape(F, S * E),
    ]
    wpk = np.ascontiguousarray(np.concatenate(blocks, axis=1)).astype(np.float32)

    # One cast of the full A, then per-core byte gathers.
    A8 = A.astype(np_a)                            # [S, n, m]
    AT = np.ascontiguousarray(A8.transpose(0, 2, 1))   # [S, m, n]

    in_maps = []
    for c in range(N_CORES):
        blk = AT[:, :, c * NS : (c + 1) * NS]      # [S, N(m), NS]
        # rows k = b*(P*JPB) + p*JPB + (h*S + s), m = (b*MPB + h)*P + p
        at2 = blk.reshape(S, NBLK, MPB, P, NS).transpose(1, 3, 2, 0, 4)
        atc = np.ascontiguousarray(at2).reshape(KTOT, NS)
        in_maps.append({"atc": atc, "featT": featT, "wpk": wpk})
    return in_maps


def _run(features, A, W, W_comp, dt_key=None, trace=False):
    adt_key = dt_key or ADT
    nc = _get_nc(adt_key, HDT)
    in_maps = _shard_inputs(features, A, W, W_comp, adt_key)
    res = bass_utils.run_bass_kernel_spmd(
        nc, in_maps, core_ids=list(range(N_CORES)), trace=trace
    )
    out = np.concatenate(
        [res.results[c]["outT"].T for c in range(N_CORES)], axis=0
    ).astype(np.float32)
    return out, res


def kernel(features, A, W, W_comp):
    try:
        out, _ = _run(features, A, W, W_comp)
    except Exception:
        # Rare transient device-unrecoverable flakes: reset jax backends and
        # retry once with a freshly built program.
        import jax
        try:
            jax.clear_caches()
            jax.extend.backend.clear_backends()
        except Exception:
            pass
        _built_cache.clear()
        out, _ = _run(features, A, W, W_comp)
    return out
